# revision 1
# baseline (speedup 1.0000x reference)
"""KMeans assignment kernel (retrieval_knn) for 8 Trainium2 NeuronCores.

Computes argmin_k ||x_n - c_k||^2 for x [262144, 64] f32 against
centers [1024, 64] f32, returning int32 cluster ids [262144].

argmin ||x-c||^2 == argmax s, s = 2x.c - ||c||^2, computed on the PE via
bf16 hi/lo split matmuls (near-fp32). ScalarE copies PSUM->SBUF; DVE does
ONE segmented-max pass (64 group maxima/tile); batched equality+iota ops
pick the winning group; scores are spilled to raw DRAM tensors and an
indirect DMA gathers just the winning 16-el group per point; a 16-wide
max_index gives the position -> id = group*16 + pos.  (Gather source must
be a raw nc.dram_tensor — DRAM pool tiles break indirect DMA.)
"""

import numpy as np
import ml_dtypes

N_POINTS = 262144
N_FEATURES = 64
N_CLUSTERS = 1024
N_CORES = 8
PTS_PER_CORE = N_POINTS // N_CORES      # 32768
TILE_P = 128                            # points per tile (partition dim)
N_TILES = PTS_PER_CORE // TILE_P        # 256
KH = 512                                # centers per matmul chunk

_CACHE = {}


def _build_bass():
    import concourse.bass as bass
    import concourse.bacc as bacc
    import concourse.mybir as mybir
    import concourse.tile as tile
    from contextlib import ExitStack

    bf16 = mybir.dt.bfloat16
    f32 = mybir.dt.float32
    u32 = mybir.dt.uint32

    nc = bacc.Bacc(None, target_bir_lowering=False)

    xpack = nc.declare_dram_parameter("xpack", [128, PTS_PER_CORE], bf16, isOutput=False)
    cc = nc.declare_dram_parameter("cc", [128, N_CLUSTERS], bf16, isOutput=False)
    cloa = nc.declare_dram_parameter("cloa", [67, N_CLUSTERS], bf16, isOutput=False)
    tc64 = nc.declare_dram_parameter("tc64", [128, 8], f32, isOutput=False)
    out = nc.declare_dram_parameter("out", [128, N_TILES], u32, isOutput=True)

    BT = 8            # tiles per stage-2 batch
    G = 64            # groups per tile
    GS = 16           # group size (elements gathered per point)

    # raw DRAM spill buffers (manual double buffer, alternating per batch)
    spills = [
        nc.dram_tensor(f"sspill{j}", [128 * BT * G, GS], f32) for j in range(2)
    ]

    with tile.TileContext(nc) as tc, ExitStack() as ctx:
        const_pool = ctx.enter_context(tc.tile_pool(name="const", bufs=1))
        xin_pool = ctx.enter_context(tc.tile_pool(name="xin", bufs=3))
        xa_pool = ctx.enter_context(tc.tile_pool(name="xa", bufs=3))
        psum_pool = ctx.enter_context(
            tc.tile_pool(name="psum", bufs=4, space=bass.MemorySpace.PSUM)
        )
        s_pool = ctx.enter_context(tc.tile_pool(name="s", bufs=4))
        batch_pool = ctx.enter_context(tc.tile_pool(name="batch", bufs=3))
        small_pool = ctx.enter_context(tc.tile_pool(name="small", bufs=6))
        gv_pool = ctx.enter_context(tc.tile_pool(name="gv", bufs=10))
        out_pool = ctx.enter_context(tc.tile_pool(name="out", bufs=1))

        cc_t = const_pool.tile([128, N_CLUSTERS], bf16)
        nc.sync.dma_start(cc_t[:], cc[:])
        cloa_t = const_pool.tile([67, N_CLUSTERS], bf16)
        nc.sync.dma_start(cloa_t[:], cloa[:])
        tc64_t = const_pool.tile([128, 8], f32)
        nc.sync.dma_start(tc64_t[:], tc64[:])

        outbuf = out_pool.tile([128, N_TILES], u32)

        XB = 4  # tiles per x load / score-spill batch
        for tb in range(N_TILES // BT):
            maB = batch_pool.tile([128, BT, G], f32)
            spillb = spills[tb % 2]
            spillb_w = spillb[:].rearrange(
                "(p i g) e -> p i (g e)", p=128, i=BT
            )
            for i in range(BT):
                t = tb * BT + i
                if t % XB == 0:
                    xp = xin_pool.tile([128, XB, TILE_P], bf16)
                    csl = slice(t * TILE_P, (t + XB) * TILE_P)
                    nc.sync.dma_start(
                        xp[:], xpack[:, csl].rearrange("p (b q) -> p b q", b=XB)
                    )
                    # second copy of the xhi rows with 3 all-ones aug rows
                    # (stationary for the xhi.clo - cn matmul)
                    xa = xa_pool.tile([67, XB, TILE_P], bf16)
                    nc.sync.dma_start(
                        xa[0:64],
                        xpack[0:64, csl].rearrange("p (b q) -> p b q", b=XB),
                    )
                    nc.gpsimd.memset(xa[64:67], 1.0)
                xi = t % XB
                ps = psum_pool.tile([128, N_CLUSTERS], f32)
                for kh in range(N_CLUSTERS // KH):
                    ksl = slice(kh * KH, (kh + 1) * KH)
                    nc.tensor.matmul(
                        ps[:, ksl], xp[:, xi, :], cc_t[:, ksl],
                        start=True, stop=False,
                    )
                    nc.tensor.matmul(
                        ps[:, ksl], xa[:, xi, :], cloa_t[:, ksl],
                        start=False, stop=True,
                    )
                if i % XB == 0:
                    s4 = s_pool.tile([128, XB, N_CLUSTERS], f32)
                si = i % XB
                for kh in range(N_CLUSTERS // KH):
                    ksl = slice(kh * KH, (kh + 1) * KH)
                    nc.scalar.copy(s4[:, si, ksl], ps[:, ksl])
                # stage 1: segmented max over 64 groups of 16
                nc.vector.tensor_reduce(
                    maB[:, i, :],
                    s4[:, si, :].rearrange("p (g e) -> p g e", g=G),
                    axis=mybir.AxisListType.X,
                    op=mybir.AluOpType.max,
                )
                if i % XB == XB - 1:
                    # spill 4 tiles of scores in one DMA, alternating the
                    # issuing queue (transfer time lands on the issuer)
                    eng = nc.gpsimd if (t // XB) % 2 == 0 else nc.sync
                    eng.dma_start(spillb_w[:, i - (XB - 1) : i + 1, :], s4[:])

            # stage 2 (batched): per-tile max value and winning group index
            m8b = small_pool.tile([128, BT], f32)
            nc.vector.tensor_reduce(
                m8b[:], maB[:], axis=mybir.AxisListType.X, op=mybir.AluOpType.max
            )
            gw = small_pool.tile([128, BT, 8], u32)
            for i in range(BT):
                nc.vector.max_index(
                    gw[:, i, :],
                    m8b[:, i : i + 1].to_broadcast([128, 8]),
                    maB[:, i, :],
                )
            g8 = small_pool.tile([128, BT], f32)
            nc.vector.tensor_copy(g8[:], gw[:, :, 0])
            # gather row index = p*(BT*G) + i*G + g  (tc64 holds the p,i part)
            offf = small_pool.tile([128, BT], f32)
            nc.vector.tensor_tensor(
                offf[:], g8[:], tc64_t[:], op=mybir.AluOpType.add
            )
            offu = small_pool.tile([128, BT], u32)
            nc.vector.tensor_copy(offu[:], offf[:])
            # stage 3: gather each tile's winning 16-el group, then find the
            # max's position within it
            jw = small_pool.tile([128, BT, 8], u32)
            for i in range(BT):
                gv = gv_pool.tile([128, GS], f32)
                nc.gpsimd.indirect_dma_start(
                    out=gv[:],
                    out_offset=None,
                    in_=spillb[:],
                    in_offset=bass.IndirectOffsetOnAxis(
                        ap=offu[:, i : i + 1], axis=0
                    ),
                )
                nc.vector.max_index(
                    jw[:, i, :],
                    m8b[:, i : i + 1].to_broadcast([128, 8]),
                    gv[:],
                )
            jf = small_pool.tile([128, BT], f32)
            nc.vector.tensor_copy(jf[:], jw[:, :, 0])
            g16 = small_pool.tile([128, BT], f32)
            nc.vector.tensor_scalar_mul(g16[:], g8[:], float(GS))
            idxf = small_pool.tile([128, BT], f32)
            nc.vector.tensor_tensor(
                idxf[:], g16[:], jf[:], op=mybir.AluOpType.add
            )
            nc.vector.tensor_copy(outbuf[:, tb * BT : (tb + 1) * BT], idxf[:])

        nc.sync.dma_start(out[:], outbuf[:])

    nc.compile()
    return nc


def _prep(x: np.ndarray, centers: np.ndarray):
    bf16 = ml_dtypes.bfloat16
    xt = np.ascontiguousarray(x.T)                      # [64, N] f32
    xhi = xt.astype(bf16)
    xlo = (xt - xhi.astype(np.float32)).astype(bf16)
    xpack = np.concatenate([xhi, xlo], axis=0)          # [128, N] bf16

    c2t = np.ascontiguousarray((2.0 * centers).T)       # [64, K] f32
    chi = c2t.astype(bf16)
    clo = (c2t - chi.astype(np.float32)).astype(bf16)   # [64, K] bf16
    cc = np.concatenate([chi, chi], axis=0)             # [128, K] bf16

    # -||c||^2 as a 3-term bf16 cascade, matched with all-ones stationary rows
    cn = np.sum(centers.astype(np.float32) ** 2, axis=1, dtype=np.float32)
    n1 = (-cn).astype(bf16)
    r1 = -cn - n1.astype(np.float32)
    n2 = r1.astype(bf16)
    n3 = (r1 - n2.astype(np.float32)).astype(bf16)
    cloa = np.concatenate(
        [clo, n1[None, :], n2[None, :], n3[None, :]], axis=0
    )                                                   # [67, K] bf16

    p = np.arange(128, dtype=np.float32)[:, None]
    i = np.arange(8, dtype=np.float32)[None, :]
    tc64 = np.ascontiguousarray(p * (8 * 64.0) + i * 64.0)
    return xpack, cc, cloa, tc64


def kernel(x: np.ndarray, centers: np.ndarray) -> np.ndarray:
    import sys
    if "/opt/trn_rl_repo" not in sys.path:
        sys.path.insert(0, "/opt/trn_rl_repo")
    from concourse.bass_utils import run_bass_kernel_spmd

    x = np.asarray(x, dtype=np.float32)
    centers = np.asarray(centers, dtype=np.float32)

    xpack, cc, cloa, tc64 = _prep(x, centers)

    if "nc" not in _CACHE:
        _CACHE["nc"] = _build_bass()
    nc = _CACHE["nc"]

    in_maps = []
    for c in range(N_CORES):
        sl = slice(c * PTS_PER_CORE, (c + 1) * PTS_PER_CORE)
        in_maps.append(
            {
                "xpack": np.ascontiguousarray(xpack[:, sl]),
                "cc": cc,
                "cloa": cloa,
                "tc64": tc64,
            }
        )

    res = run_bass_kernel_spmd(nc, in_maps, list(range(N_CORES)))

    outs = []
    for c in range(N_CORES):
        o = res.results[c]["out"]                       # [128, N_TILES] uint32
        outs.append(np.asarray(o).astype(np.int64).T.reshape(-1))  # point t*128+p
    ids = np.concatenate(outs)
    return ids.astype(np.int32)


if __name__ == "__main__":
    rng = np.random.default_rng(0)
    x = rng.normal(size=(N_POINTS, N_FEATURES)).astype(np.float32)
    c = rng.normal(size=(N_CLUSTERS, N_FEATURES)).astype(np.float32)
    ids = kernel(x=x, centers=c)
    d = (
        np.sum(x * x, 1)[:, None]
        - 2.0 * (x @ c.T)
        + np.sum(c * c, 1)[None, :]
    )
    ref = np.argmin(np.abs(d), axis=1)
    print("mismatch:", np.mean(ids != ref))



# revision 10
# speedup vs baseline: 1.2506x; 1.2506x over previous
"""KMeans assignment kernel (retrieval_knn) for 8 Trainium2 NeuronCores.

Computes argmin_k ||x_n - c_k||^2 for x [262144, 64] f32 against centers
[1024, 64] f32, returning int32 cluster ids [262144].

argmin ||x-c||^2 == argmax s, s = 2x.c - ||c||^2, computed on the PE via
bf16 hi/lo split matmuls (near-fp32, same scheme as the reference's fp32
einsum to ~1e-5).  The entire argmax (max + index) is then ONE custom DVE
instruction per tile (ARGMAX_ANT, registered at build time into the
per-NEFF DVE table): running scan-MAX + eq + select(Idx) + accum-MAX
returns the argmax position directly from fp32 PSUM.  No score spill, no
gather, no PSUM->SBUF copies, no multi-instruction reduce cascades.
"""

import numpy as np
import ml_dtypes

N_POINTS = 262144
N_FEATURES = 64
N_CLUSTERS = 1024
N_CORES = 8
PTS_PER_CORE = N_POINTS // N_CORES      # 32768
TILE_P = 128                            # points per tile (partition dim)
N_TILES = PTS_PER_CORE // TILE_P        # 256
KH = 512                                # centers per matmul chunk
BT = 8                                  # tiles per output batch

_CACHE = {}


def _register_argmax_op():
    """Register the custom ARGMAX_ANT DVE op (runtime append to dve_ops.OPS).

    accum_out[p] = max_k select(in0[p,k] == runmax(in0)[p,k], k, -FLT_MAX)
                 = argmax_k in0[p,k]   (last tie wins; exact fp32 ties are
                   vanishingly rare for these scores)
    """
    from concourse import dve_ops
    from concourse.dve_spec import (
        Spec, Src0, Idx, MaxNeg, AluOp, scan, eq, select, maxx,
    )

    if "ARGMAX_ANT" in dve_ops._SUB_OPCODE_FOR_NAME:
        return dve_ops.CUSTOM_DVE_SPECS["ARGMAX_ANT"] and _CACHE["argmax_op"]

    def _ref_argmax(in0, in1, s0, s1, imm2):
        r = np.maximum.accumulate(in0, axis=-1)
        idx = np.arange(in0.shape[-1], dtype=np.float32)
        return np.where(in0 == r, idx, -np.finfo(np.float32).max)

    op = dve_ops.DveOp(
        "ARGMAX_ANT",
        Spec(
            body=select(eq(Src0, scan(AluOp.MAX, Src0)), Idx, MaxNeg),
            accum=maxx,
            reference=_ref_argmax,
        ),
        subdim=False,
        uops_sha={"v3": "d14dbf28477fed0e", "v4": "7311a447fa794d46"},
    )
    dve_ops.OPS.append(op)
    dve_ops._SUB_OPCODE_FOR_NAME["ARGMAX_ANT"] = (
        dve_ops._CUSTOM_DVE_ROW_BASE + len(dve_ops.OPS) - 1
    )
    dve_ops.CUSTOM_DVE_SPECS["ARGMAX_ANT"] = op.spec
    _CACHE["argmax_op"] = op
    return op


def _build_bass():
    import concourse.bass as bass
    import concourse.bacc as bacc
    import concourse.mybir as mybir
    import concourse.tile as tile
    from contextlib import ExitStack

    argmax_op = _register_argmax_op()

    bf16 = mybir.dt.bfloat16
    f32 = mybir.dt.float32
    u32 = mybir.dt.uint32

    nc = bacc.Bacc(None, target_bir_lowering=False)

    xpack = nc.declare_dram_parameter("xpack", [128, PTS_PER_CORE], bf16, isOutput=False)
    xa = nc.declare_dram_parameter("xa", [67, PTS_PER_CORE], bf16, isOutput=False)
    cc = nc.declare_dram_parameter("cc", [128, N_CLUSTERS], bf16, isOutput=False)
    cloa = nc.declare_dram_parameter("cloa", [67, N_CLUSTERS], bf16, isOutput=False)
    out = nc.declare_dram_parameter("out", [128, N_TILES], u32, isOutput=True)

    with tile.TileContext(nc) as tc, ExitStack() as ctx:
        const_pool = ctx.enter_context(tc.tile_pool(name="const", bufs=1))
        psum_pool = ctx.enter_context(
            tc.tile_pool(name="psum", bufs=4, space=bass.MemorySpace.PSUM)
        )
        scr_pool = ctx.enter_context(tc.tile_pool(name="scr", bufs=4))
        idx_pool = ctx.enter_context(tc.tile_pool(name="idx", bufs=3))
        out_pool = ctx.enter_context(tc.tile_pool(name="out", bufs=1))

        cc_t = const_pool.tile([128, N_CLUSTERS], bf16)
        nc.sync.dma_start(cc_t[:], cc[:])
        cloa_t = const_pool.tile([67, N_CLUSTERS], bf16)
        nc.sync.dma_start(cloa_t[:], cloa[:])
        # resident stationary inputs, chunked loads so tile 0 can start early
        xpack_t = const_pool.tile([128, PTS_PER_CORE], bf16)
        xa_t = const_pool.tile([67, PTS_PER_CORE], bf16)
        XCH = 8
        CHW = PTS_PER_CORE // XCH
        for ch in range(XCH):
            csl = slice(ch * CHW, (ch + 1) * CHW)
            nc.sync.dma_start(xpack_t[:, csl], xpack[:, csl])
            nc.sync.dma_start(xa_t[:, csl], xa[:, csl])

        outbuf = out_pool.tile([128, N_TILES], u32)

        for tb in range(N_TILES // BT):
            idxb = idx_pool.tile([128, BT], f32)
            for i in range(BT):
                t = tb * BT + i
                tsl = slice(t * TILE_P, (t + 1) * TILE_P)
                ps = psum_pool.tile([128, N_CLUSTERS], f32)
                for kh in range(N_CLUSTERS // KH):
                    ksl = slice(kh * KH, (kh + 1) * KH)
                    nc.tensor.matmul(
                        ps[:, ksl], xpack_t[:, tsl], cc_t[:, ksl],
                        start=True, stop=False,
                    )
                    nc.tensor.matmul(
                        ps[:, ksl], xa_t[:, tsl], cloa_t[:, ksl],
                        start=False, stop=True,
                    )
                scratch = scr_pool.tile([128, N_CLUSTERS], f32)
                nc.vector._custom_dve(
                    argmax_op,
                    out=scratch[:],
                    in0=ps[:],
                    accum_out=idxb[:, i : i + 1],
                )
            nc.vector.tensor_copy(outbuf[:, tb * BT : (tb + 1) * BT], idxb[:])

        nc.sync.dma_start(out[:], outbuf[:])

    nc.compile()
    return nc


def _prep(x: np.ndarray, centers: np.ndarray):
    bf16 = ml_dtypes.bfloat16
    xt = np.ascontiguousarray(x.T)                      # [64, N] f32
    xhi = xt.astype(bf16)
    xlo = (xt - xhi.astype(np.float32)).astype(bf16)
    xpack = np.concatenate([xhi, xlo], axis=0)          # [128, N] bf16
    xa = np.empty((67, x.shape[0]), bf16)               # xhi + 3 ones rows
    xa[0:64] = xhi
    xa[64:67] = bf16(1.0)

    c2t = np.ascontiguousarray((2.0 * centers).T)       # [64, K] f32
    chi = c2t.astype(bf16)
    clo = (c2t - chi.astype(np.float32)).astype(bf16)   # [64, K] bf16
    cc = np.concatenate([chi, chi], axis=0)             # [128, K] bf16

    # -||c||^2 as a 3-term bf16 cascade, matched with the ones rows of xa
    cn = np.sum(centers.astype(np.float64) ** 2, axis=1)
    n1 = (-cn).astype(bf16)
    r1 = -cn - n1.astype(np.float64)
    n2 = r1.astype(bf16)
    n3 = (r1 - n2.astype(np.float64)).astype(bf16)
    cloa = np.concatenate(
        [clo, n1[None, :], n2[None, :], n3[None, :]], axis=0
    )                                                   # [67, K] bf16
    return xpack, xa, cc, cloa


def kernel(x: np.ndarray, centers: np.ndarray) -> np.ndarray:
    import sys
    if "/opt/trn_rl_repo" not in sys.path:
        sys.path.insert(0, "/opt/trn_rl_repo")
    from concourse.bass_utils import run_bass_kernel_spmd

    x = np.asarray(x, dtype=np.float32)
    centers = np.asarray(centers, dtype=np.float32)

    xpack, xa, cc, cloa = _prep(x, centers)

    if "nc" not in _CACHE:
        _CACHE["nc"] = _build_bass()
    nc = _CACHE["nc"]

    in_maps = []
    for c in range(N_CORES):
        sl = slice(c * PTS_PER_CORE, (c + 1) * PTS_PER_CORE)
        in_maps.append(
            {
                "xpack": np.ascontiguousarray(xpack[:, sl]),
                "xa": np.ascontiguousarray(xa[:, sl]),
                "cc": cc,
                "cloa": cloa,
            }
        )

    res = run_bass_kernel_spmd(nc, in_maps, list(range(N_CORES)))

    outs = []
    for c in range(N_CORES):
        o = res.results[c]["out"]                       # [128, N_TILES] uint32
        outs.append(np.asarray(o).astype(np.int64).T.reshape(-1))  # point t*128+p
    ids = np.concatenate(outs)
    return ids.astype(np.int32)


if __name__ == "__main__":
    rng = np.random.default_rng(0)
    x = rng.normal(size=(N_POINTS, N_FEATURES)).astype(np.float32)
    c = rng.normal(size=(N_CLUSTERS, N_FEATURES)).astype(np.float32)
    ids = kernel(x=x, centers=c)
    d = (
        np.sum(x * x, 1)[:, None]
        - 2.0 * (x @ c.T)
        + np.sum(c * c, 1)[None, :]
    )
    ref = np.argmin(np.abs(d), axis=1)
    print("mismatch:", np.mean(ids != ref))


# revision 14
# speedup vs baseline: 1.2732x; 1.0181x over previous
"""KMeans assignment kernel (retrieval_knn) for 8 Trainium2 NeuronCores.

Computes argmin_k ||x_n - c_k||^2 for x [262144, 64] f32 against centers
[1024, 64] f32, returning int32 cluster ids [262144].

argmin ||x-c||^2 == argmax s, s = 2x.c - ||c||^2, computed on the PE via
bf16 hi/lo split matmuls (near-fp32, same scheme as the reference's fp32
einsum to ~1e-5).  The entire argmax (max + index) is then ONE custom DVE
instruction per tile (ARGMAX_ANT, registered at build time into the
per-NEFF DVE table): running scan-MAX + eq + select(Idx) + accum-MAX
returns the argmax position directly from fp32 PSUM.  No score spill, no
gather, no PSUM->SBUF copies, no multi-instruction reduce cascades.
"""

import numpy as np
import ml_dtypes

N_POINTS = 262144
N_FEATURES = 64
N_CLUSTERS = 1024
N_CORES = 8
PTS_PER_CORE = N_POINTS // N_CORES      # 32768
TILE_P = 128                            # points per tile (partition dim)
N_TILES = PTS_PER_CORE // TILE_P        # 256
KH = 512                                # centers per matmul chunk
BT = 8                                  # tiles per output batch

_CACHE = {}


def _register_argmax_op():
    """Register the custom ARGMAX_ANT DVE op (runtime append to dve_ops.OPS).

    accum_out[p] = max_k select(in0[p,k] == runmax(in0)[p,k], k, -FLT_MAX)
                 = argmax_k in0[p,k]   (last tie wins; exact fp32 ties are
                   vanishingly rare for these scores)
    """
    from concourse import dve_ops
    from concourse.dve_spec import (
        Spec, Src0, Idx, MaxNeg, AluOp, scan, eq, select, maxx,
    )

    if "ARGMAX_ANT" in dve_ops._SUB_OPCODE_FOR_NAME:
        return _CACHE["argmax_op"]

    def _ref_argmax(in0, in1, s0, s1, imm2):
        r = np.maximum.accumulate(in0, axis=-1)
        idx = np.arange(in0.shape[-1], dtype=np.float32)
        return np.where(in0 == r, idx, -np.finfo(np.float32).max)

    op = dve_ops.DveOp(
        "ARGMAX_ANT",
        Spec(
            body=select(eq(Src0, scan(AluOp.MAX, Src0)), Idx, MaxNeg),
            accum=maxx,
            reference=_ref_argmax,
        ),
        subdim=False,
        uops_sha={"v3": "d14dbf28477fed0e", "v4": "7311a447fa794d46"},
    )
    dve_ops.OPS.append(op)
    dve_ops._SUB_OPCODE_FOR_NAME["ARGMAX_ANT"] = (
        dve_ops._CUSTOM_DVE_ROW_BASE + len(dve_ops.OPS) - 1
    )
    dve_ops.CUSTOM_DVE_SPECS["ARGMAX_ANT"] = op.spec
    _CACHE["argmax_op"] = op
    return op


def _build_bass():
    import concourse.bass as bass
    import concourse.bacc as bacc
    import concourse.mybir as mybir
    import concourse.tile as tile
    from contextlib import ExitStack

    argmax_op = _register_argmax_op()

    bf16 = mybir.dt.bfloat16
    f32 = mybir.dt.float32
    u32 = mybir.dt.uint32

    nc = bacc.Bacc(None, target_bir_lowering=False)

    xpack = nc.declare_dram_parameter("xpack", [128, PTS_PER_CORE], bf16, isOutput=False)
    xa = nc.declare_dram_parameter("xa", [67, PTS_PER_CORE], bf16, isOutput=False)
    cc = nc.declare_dram_parameter("cc", [128, N_CLUSTERS], bf16, isOutput=False)
    cloa = nc.declare_dram_parameter("cloa", [67, N_CLUSTERS], bf16, isOutput=False)
    out = nc.declare_dram_parameter("out", [128, N_TILES], u32, isOutput=True)

    with tile.TileContext(nc) as tc, ExitStack() as ctx:
        const_pool = ctx.enter_context(tc.tile_pool(name="const", bufs=1))
        psum_pool = ctx.enter_context(
            tc.tile_pool(name="psum", bufs=4, space=bass.MemorySpace.PSUM)
        )
        scr_pool = ctx.enter_context(tc.tile_pool(name="scr", bufs=4))
        idx_pool = ctx.enter_context(tc.tile_pool(name="idx", bufs=3))
        out_pool = ctx.enter_context(tc.tile_pool(name="out", bufs=1))

        cc_t = const_pool.tile([128, N_CLUSTERS], bf16)
        nc.sync.dma_start(cc_t[:], cc[:])
        cloa_t = const_pool.tile([67, N_CLUSTERS], bf16)
        nc.gpsimd.dma_start(cloa_t[:], cloa[:])
        # resident stationary inputs; chunked loads on two independent DMA
        # queues (sync for xpack, gpsimd for xa) so tile 0 starts early
        xpack_t = const_pool.tile([128, PTS_PER_CORE], bf16)
        xa_t = const_pool.tile([67, PTS_PER_CORE], bf16)
        XCH = 16
        CHW = PTS_PER_CORE // XCH
        for ch in range(XCH):
            csl = slice(ch * CHW, (ch + 1) * CHW)
            nc.sync.dma_start(xpack_t[:, csl], xpack[:, csl])
            nc.gpsimd.dma_start(xa_t[:, csl], xa[:, csl])

        outbuf = out_pool.tile([128, N_TILES], u32)

        for tb in range(N_TILES // BT):
            idxb = idx_pool.tile([128, BT], f32)
            for i in range(BT):
                t = tb * BT + i
                tsl = slice(t * TILE_P, (t + 1) * TILE_P)
                ps = psum_pool.tile([128, N_CLUSTERS], f32)
                for kh in range(N_CLUSTERS // KH):
                    ksl = slice(kh * KH, (kh + 1) * KH)
                    nc.tensor.matmul(
                        ps[:, ksl], xpack_t[:, tsl], cc_t[:, ksl],
                        start=True, stop=False,
                    )
                    nc.tensor.matmul(
                        ps[:, ksl], xa_t[:, tsl], cloa_t[:, ksl],
                        start=False, stop=True,
                    )
                scratch = scr_pool.tile([128, N_CLUSTERS], f32)
                nc.vector._custom_dve(
                    argmax_op,
                    out=scratch[:],
                    in0=ps[:],
                    accum_out=idxb[:, i : i + 1],
                )
            nc.scalar.copy(outbuf[:, tb * BT : (tb + 1) * BT], idxb[:])

        nc.sync.dma_start(out[:], outbuf[:])

    nc.compile()
    return nc


def _prep(x: np.ndarray, centers: np.ndarray):
    bf16 = ml_dtypes.bfloat16
    xt = np.ascontiguousarray(x.T)                      # [64, N] f32
    xhi = xt.astype(bf16)
    xlo = (xt - xhi.astype(np.float32)).astype(bf16)
    xpack = np.concatenate([xhi, xlo], axis=0)          # [128, N] bf16
    xa = np.empty((67, x.shape[0]), bf16)               # xhi + 3 ones rows
    xa[0:64] = xhi
    xa[64:67] = bf16(1.0)

    c2t = np.ascontiguousarray((2.0 * centers).T)       # [64, K] f32
    chi = c2t.astype(bf16)
    clo = (c2t - chi.astype(np.float32)).astype(bf16)   # [64, K] bf16
    cc = np.concatenate([chi, chi], axis=0)             # [128, K] bf16

    # -||c||^2 as a 3-term bf16 cascade, matched with the ones rows of xa
    cn = np.sum(centers.astype(np.float64) ** 2, axis=1)
    n1 = (-cn).astype(bf16)
    r1 = -cn - n1.astype(np.float64)
    n2 = r1.astype(bf16)
    n3 = (r1 - n2.astype(np.float64)).astype(bf16)
    cloa = np.concatenate(
        [clo, n1[None, :], n2[None, :], n3[None, :]], axis=0
    )                                                   # [67, K] bf16
    return xpack, xa, cc, cloa


def kernel(x: np.ndarray, centers: np.ndarray) -> np.ndarray:
    import sys
    if "/opt/trn_rl_repo" not in sys.path:
        sys.path.insert(0, "/opt/trn_rl_repo")
    from concourse.bass_utils import run_bass_kernel_spmd

    x = np.asarray(x, dtype=np.float32)
    centers = np.asarray(centers, dtype=np.float32)

    xpack, xa, cc, cloa = _prep(x, centers)

    if "nc" not in _CACHE:
        _CACHE["nc"] = _build_bass()
    nc = _CACHE["nc"]

    in_maps = []
    for c in range(N_CORES):
        sl = slice(c * PTS_PER_CORE, (c + 1) * PTS_PER_CORE)
        in_maps.append(
            {
                "xpack": np.ascontiguousarray(xpack[:, sl]),
                "xa": np.ascontiguousarray(xa[:, sl]),
                "cc": cc,
                "cloa": cloa,
            }
        )

    res = run_bass_kernel_spmd(nc, in_maps, list(range(N_CORES)))

    outs = []
    for c in range(N_CORES):
        o = res.results[c]["out"]                       # [128, N_TILES] uint32
        outs.append(np.asarray(o).astype(np.int64).T.reshape(-1))  # point t*128+p
    ids = np.concatenate(outs)
    return ids.astype(np.int32)


if __name__ == "__main__":
    rng = np.random.default_rng(0)
    x = rng.normal(size=(N_POINTS, N_FEATURES)).astype(np.float32)
    c = rng.normal(size=(N_CLUSTERS, N_FEATURES)).astype(np.float32)
    ids = kernel(x=x, centers=c)
    d = (
        np.sum(x * x, 1)[:, None]
        - 2.0 * (x @ c.T)
        + np.sum(c * c, 1)[None, :]
    )
    ref = np.argmin(np.abs(d), axis=1)
    print("mismatch:", np.mean(ids != ref))


# revision 17
# speedup vs baseline: 1.2772x; 1.0031x over previous
"""KMeans assignment kernel (retrieval_knn) for 8 Trainium2 NeuronCores.

Computes argmin_k ||x_n - c_k||^2 for x [262144, 64] f32 against centers
[1024, 64] f32, returning int32 cluster ids [262144].

argmin ||x-c||^2 == argmax s, s = 2x.c - ||c||^2, computed on the PE via
bf16 hi/lo split matmuls (near-fp32, same scheme as the reference's fp32
einsum to ~1e-5).  The entire argmax (max + index) is then ONE custom DVE
instruction per tile (ARGMAX_ANT, registered at build time into the
per-NEFF DVE table): running scan-MAX + eq + select(Idx) + accum-MAX
returns the argmax position directly from fp32 PSUM.  No score spill, no
gather, no PSUM->SBUF copies, no multi-instruction reduce cascades.
"""

import numpy as np
import ml_dtypes

N_POINTS = 262144
N_FEATURES = 64
N_CLUSTERS = 1024
N_CORES = 8
PTS_PER_CORE = N_POINTS // N_CORES      # 32768
TILE_P = 128                            # points per tile (partition dim)
N_TILES = PTS_PER_CORE // TILE_P        # 256
KH = 512                                # centers per matmul chunk
BT = 8                                  # tiles per output batch

_CACHE = {}


def _register_argmax_op():
    """Register the custom ARGMAX_ANT DVE op (runtime append to dve_ops.OPS).

    accum_out[p] = max_k select(in0[p,k] == runmax(in0)[p,k], k, -FLT_MAX)
                 = argmax_k in0[p,k]   (last tie wins; exact fp32 ties are
                   vanishingly rare for these scores)
    """
    from concourse import dve_ops
    from concourse.dve_spec import (
        Spec, Src0, Idx, MaxNeg, AluOp, scan, eq, select, maxx,
    )

    if "ARGMAX_ANT" in dve_ops._SUB_OPCODE_FOR_NAME:
        return _CACHE["argmax_op"]

    def _ref_argmax(in0, in1, s0, s1, imm2):
        r = np.maximum.accumulate(in0, axis=-1)
        idx = np.arange(in0.shape[-1], dtype=np.float32)
        return np.where(in0 == r, idx, -np.finfo(np.float32).max)

    op = dve_ops.DveOp(
        "ARGMAX_ANT",
        Spec(
            body=select(eq(Src0, scan(AluOp.MAX, Src0)), Idx, MaxNeg),
            accum=maxx,
            reference=_ref_argmax,
        ),
        subdim=False,
        uops_sha={"v3": "d14dbf28477fed0e", "v4": "7311a447fa794d46"},
    )
    dve_ops.OPS.append(op)
    dve_ops._SUB_OPCODE_FOR_NAME["ARGMAX_ANT"] = (
        dve_ops._CUSTOM_DVE_ROW_BASE + len(dve_ops.OPS) - 1
    )
    dve_ops.CUSTOM_DVE_SPECS["ARGMAX_ANT"] = op.spec
    _CACHE["argmax_op"] = op
    return op


def _build_bass():
    import concourse.bass as bass
    import concourse.bacc as bacc
    import concourse.mybir as mybir
    import concourse.tile as tile
    from contextlib import ExitStack

    argmax_op = _register_argmax_op()

    bf16 = mybir.dt.bfloat16
    f32 = mybir.dt.float32
    u32 = mybir.dt.uint32

    nc = bacc.Bacc(None, target_bir_lowering=False)

    xpack = nc.declare_dram_parameter("xpack", [128, PTS_PER_CORE], bf16, isOutput=False)
    xa = nc.declare_dram_parameter("xa", [67, PTS_PER_CORE], bf16, isOutput=False)
    cc = nc.declare_dram_parameter("cc", [128, N_CLUSTERS], bf16, isOutput=False)
    cloa = nc.declare_dram_parameter("cloa", [67, N_CLUSTERS], bf16, isOutput=False)
    out = nc.declare_dram_parameter("out", [128, N_TILES], u32, isOutput=True)

    with tile.TileContext(nc) as tc, ExitStack() as ctx:
        const_pool = ctx.enter_context(tc.tile_pool(name="const", bufs=1))
        psum_pool = ctx.enter_context(
            tc.tile_pool(name="psum", bufs=4, space=bass.MemorySpace.PSUM)
        )
        scr_pool = ctx.enter_context(tc.tile_pool(name="scr", bufs=4))
        idx_pool = ctx.enter_context(tc.tile_pool(name="idx", bufs=3))
        out_pool = ctx.enter_context(tc.tile_pool(name="out", bufs=1))

        cc_t = const_pool.tile([128, N_CLUSTERS], bf16)
        nc.sync.dma_start(cc_t[:], cc[:])
        cloa_t = const_pool.tile([67, N_CLUSTERS], bf16)
        nc.gpsimd.dma_start(cloa_t[:], cloa[:])
        # resident stationary inputs; chunked loads on two independent DMA
        # queues (sync for xpack, gpsimd for xa) so tile 0 starts early
        xpack_t = const_pool.tile([128, PTS_PER_CORE], bf16)
        xa_t = const_pool.tile([67, PTS_PER_CORE], bf16)
        XCH = 32
        CHW = PTS_PER_CORE // XCH
        for ch in range(XCH):
            csl = slice(ch * CHW, (ch + 1) * CHW)
            nc.sync.dma_start(xpack_t[:, csl], xpack[:, csl])
            nc.gpsimd.dma_start(xa_t[:, csl], xa[:, csl])

        # warm the PE p-state during the x-load dead time: dummy matmuls on
        # the already-resident centers table so the first real tiles run at
        # full clock instead of paying the 3us ramp (tile is never read; it
        # rotates back into the pool and start=True resets the banks)
        ps = psum_pool.tile([128, N_CLUSTERS], f32)
        for _ in range(8):
            nc.tensor.matmul(
                ps[:, 0:KH], cc_t[:, 0:TILE_P], cc_t[:, 0:KH], start=True, stop=True
            )

        outbuf = out_pool.tile([128, N_TILES], u32)

        for tb in range(N_TILES // BT):
            idxb = idx_pool.tile([128, BT], f32)
            for i in range(BT):
                t = tb * BT + i
                tsl = slice(t * TILE_P, (t + 1) * TILE_P)
                ps = psum_pool.tile([128, N_CLUSTERS], f32)
                for kh in range(N_CLUSTERS // KH):
                    ksl = slice(kh * KH, (kh + 1) * KH)
                    nc.tensor.matmul(
                        ps[:, ksl], xpack_t[:, tsl], cc_t[:, ksl],
                        start=True, stop=False,
                    )
                    nc.tensor.matmul(
                        ps[:, ksl], xa_t[:, tsl], cloa_t[:, ksl],
                        start=False, stop=True,
                    )
                scratch = scr_pool.tile([128, N_CLUSTERS], f32)
                nc.vector._custom_dve(
                    argmax_op,
                    out=scratch[:],
                    in0=ps[:],
                    accum_out=idxb[:, i : i + 1],
                )
            nc.scalar.copy(outbuf[:, tb * BT : (tb + 1) * BT], idxb[:])

        nc.sync.dma_start(out[:], outbuf[:])

    nc.compile()
    return nc


def _prep(x: np.ndarray, centers: np.ndarray):
    bf16 = ml_dtypes.bfloat16
    xt = np.ascontiguousarray(x.T)                      # [64, N] f32
    xhi = xt.astype(bf16)
    xlo = (xt - xhi.astype(np.float32)).astype(bf16)
    xpack = np.concatenate([xhi, xlo], axis=0)          # [128, N] bf16
    xa = np.empty((67, x.shape[0]), bf16)               # xhi + 3 ones rows
    xa[0:64] = xhi
    xa[64:67] = bf16(1.0)

    c2t = np.ascontiguousarray((2.0 * centers).T)       # [64, K] f32
    chi = c2t.astype(bf16)
    clo = (c2t - chi.astype(np.float32)).astype(bf16)   # [64, K] bf16
    cc = np.concatenate([chi, chi], axis=0)             # [128, K] bf16

    # -||c||^2 as a 3-term bf16 cascade, matched with the ones rows of xa
    cn = np.sum(centers.astype(np.float64) ** 2, axis=1)
    n1 = (-cn).astype(bf16)
    r1 = -cn - n1.astype(np.float64)
    n2 = r1.astype(bf16)
    n3 = (r1 - n2.astype(np.float64)).astype(bf16)
    cloa = np.concatenate(
        [clo, n1[None, :], n2[None, :], n3[None, :]], axis=0
    )                                                   # [67, K] bf16
    return xpack, xa, cc, cloa


def kernel(x: np.ndarray, centers: np.ndarray) -> np.ndarray:
    import sys
    if "/opt/trn_rl_repo" not in sys.path:
        sys.path.insert(0, "/opt/trn_rl_repo")
    from concourse.bass_utils import run_bass_kernel_spmd

    x = np.asarray(x, dtype=np.float32)
    centers = np.asarray(centers, dtype=np.float32)

    xpack, xa, cc, cloa = _prep(x, centers)

    if "nc" not in _CACHE:
        _CACHE["nc"] = _build_bass()
    nc = _CACHE["nc"]

    in_maps = []
    for c in range(N_CORES):
        sl = slice(c * PTS_PER_CORE, (c + 1) * PTS_PER_CORE)
        in_maps.append(
            {
                "xpack": np.ascontiguousarray(xpack[:, sl]),
                "xa": np.ascontiguousarray(xa[:, sl]),
                "cc": cc,
                "cloa": cloa,
            }
        )

    res = run_bass_kernel_spmd(nc, in_maps, list(range(N_CORES)))

    outs = []
    for c in range(N_CORES):
        o = res.results[c]["out"]                       # [128, N_TILES] uint32
        outs.append(np.asarray(o).astype(np.int64).T.reshape(-1))  # point t*128+p
    ids = np.concatenate(outs)
    return ids.astype(np.int32)


if __name__ == "__main__":
    rng = np.random.default_rng(0)
    x = rng.normal(size=(N_POINTS, N_FEATURES)).astype(np.float32)
    c = rng.normal(size=(N_CLUSTERS, N_FEATURES)).astype(np.float32)
    ids = kernel(x=x, centers=c)
    d = (
        np.sum(x * x, 1)[:, None]
        - 2.0 * (x @ c.T)
        + np.sum(c * c, 1)[None, :]
    )
    ref = np.argmin(np.abs(d), axis=1)
    print("mismatch:", np.mean(ids != ref))


# revision 24
# speedup vs baseline: 1.3304x; 1.0417x over previous
"""KMeans assignment kernel (retrieval_knn) for 8 Trainium2 NeuronCores.

Computes argmin_k ||x_n - c_k||^2 for x [262144, 64] f32 against centers
[1024, 64] f32, returning int32 cluster ids [262144].

argmin ||x-c||^2 == argmax s, s = 2x.c - ||c||^2, computed on the PE via
bf16 hi/lo split matmuls (near-fp32, same scheme as the reference's fp32
einsum to ~1e-5).  The entire argmax (max + index) is then ONE custom DVE
instruction per tile (ARGMAX_ANT, registered at build time into the
per-NEFF DVE table): running scan-MAX + eq + select(Idx) + accum-MAX
returns the argmax position directly from fp32 PSUM.  No score spill, no
gather, no PSUM->SBUF copies, no multi-instruction reduce cascades.
"""

import numpy as np
import ml_dtypes

N_POINTS = 262144
N_FEATURES = 64
N_CLUSTERS = 1024
N_CORES = 8
PTS_PER_CORE = N_POINTS // N_CORES      # 32768
TILE_P = 128                            # points per tile (partition dim)
N_TILES = PTS_PER_CORE // TILE_P        # 256
KH = 512                                # centers per matmul chunk
BT = 8                                  # tiles per output batch

_CACHE = {}


def _register_argmax_op():
    """Register the custom ARGMAX_ANT DVE op (runtime append to dve_ops.OPS).

    accum_out[p] = max_k select(in0[p,k] == runmax(in0)[p,k], k, -FLT_MAX)
                 = argmax_k in0[p,k]   (last tie wins; exact fp32 ties are
                   vanishingly rare for these scores)
    """
    from concourse import dve_ops
    from concourse.dve_spec import (
        Spec, Src0, Idx, MaxNeg, AluOp, scan, eq, select, maxx,
    )

    if "ARGMAX_ANT" in dve_ops._SUB_OPCODE_FOR_NAME:
        return _CACHE["argmax_op"]

    def _ref_argmax(in0, in1, s0, s1, imm2):
        r = np.maximum.accumulate(in0, axis=-1)
        idx = np.arange(in0.shape[-1], dtype=np.float32)
        return np.where(in0 == r, idx, -np.finfo(np.float32).max)

    op = dve_ops.DveOp(
        "ARGMAX_ANT",
        Spec(
            body=select(eq(Src0, scan(AluOp.MAX, Src0)), Idx, MaxNeg),
            accum=maxx,
            reference=_ref_argmax,
        ),
        subdim=False,
        uops_sha={"v3": "d14dbf28477fed0e", "v4": "7311a447fa794d46"},
    )
    dve_ops.OPS.append(op)
    dve_ops._SUB_OPCODE_FOR_NAME["ARGMAX_ANT"] = (
        dve_ops._CUSTOM_DVE_ROW_BASE + len(dve_ops.OPS) - 1
    )
    dve_ops.CUSTOM_DVE_SPECS["ARGMAX_ANT"] = op.spec
    _CACHE["argmax_op"] = op
    return op


def _build_bass():
    import concourse.bass as bass
    import concourse.bacc as bacc
    import concourse.mybir as mybir
    import concourse.tile as tile
    from contextlib import ExitStack

    argmax_op = _register_argmax_op()

    bf16 = mybir.dt.bfloat16
    f32 = mybir.dt.float32
    u32 = mybir.dt.uint32

    nc = bacc.Bacc(None, target_bir_lowering=False)

    xpack = nc.declare_dram_parameter("xpack", [128, PTS_PER_CORE], bf16, isOutput=False)
    xa = nc.declare_dram_parameter("xa", [67, PTS_PER_CORE], bf16, isOutput=False)
    cc = nc.declare_dram_parameter("cc", [128, N_CLUSTERS], bf16, isOutput=False)
    cloa = nc.declare_dram_parameter("cloa", [67, N_CLUSTERS], bf16, isOutput=False)
    out = nc.declare_dram_parameter("out", [128, N_TILES], u32, isOutput=True)

    with tile.TileContext(nc) as tc, ExitStack() as ctx:
        const_pool = ctx.enter_context(tc.tile_pool(name="const", bufs=1))
        psum_pool = ctx.enter_context(
            tc.tile_pool(name="psum", bufs=2, space=bass.MemorySpace.PSUM)
        )
        scr_pool = ctx.enter_context(tc.tile_pool(name="scr", bufs=3))
        stage_pool = ctx.enter_context(tc.tile_pool(name="stage", bufs=4))
        idx_pool = ctx.enter_context(tc.tile_pool(name="idx", bufs=3))
        out_pool = ctx.enter_context(tc.tile_pool(name="out", bufs=1))

        cc_t = const_pool.tile([128, N_CLUSTERS], bf16)
        nc.sync.dma_start(cc_t[:], cc[:])
        cloa_t = const_pool.tile([67, N_CLUSTERS], bf16)
        nc.gpsimd.dma_start(cloa_t[:], cloa[:])
        # resident stationary inputs; chunked loads on two independent DMA
        # queues (sync for xpack, gpsimd for xa) so tile 0 starts early
        xpack_t = const_pool.tile([128, PTS_PER_CORE], bf16)
        xa_t = const_pool.tile([67, PTS_PER_CORE], bf16)
        XCH = 32
        CHW = PTS_PER_CORE // XCH
        for ch in range(XCH):
            csl = slice(ch * CHW, (ch + 1) * CHW)
            nc.sync.dma_start(xpack_t[:, csl], xpack[:, csl])
            nc.gpsimd.dma_start(xa_t[:, csl], xa[:, csl])

        # warm the PE p-state during the x-load dead time: dummy matmuls on
        # the already-resident centers table so the first real tiles run at
        # full clock instead of paying the 3us ramp (tile is never read; it
        # rotates back into the pool and start=True resets the banks)
        ps2 = psum_pool.tile([128, 2, N_CLUSTERS], f32)
        for _ in range(8):
            nc.tensor.matmul(
                ps2[:, 0, 0:KH], cc_t[:, 0:TILE_P], cc_t[:, 0:KH],
                start=True, stop=True,
            )

        outbuf = out_pool.tile([128, N_TILES], u32)

        # pair-wise pipeline: PE fills a 2-tile PSUM pair, the otherwise-idle
        # ACT engine stages it to SBUF (one 2048-elem copy), DVE argmaxes the
        # two SBUF slices (58-cycle SBUF access instead of 120-cycle PSUM)
        for m in range(N_TILES // 2):
            if m % (BT // 2) == 0:
                idxb = idx_pool.tile([128, BT], f32)
            ps2 = psum_pool.tile([128, 2, N_CLUSTERS], f32)
            for i in range(2):
                t = 2 * m + i
                tsl = slice(t * TILE_P, (t + 1) * TILE_P)
                for kh in range(N_CLUSTERS // KH):
                    ksl = slice(kh * KH, (kh + 1) * KH)
                    nc.tensor.matmul(
                        ps2[:, i, ksl], xpack_t[:, tsl], cc_t[:, ksl],
                        start=True, stop=False,
                    )
                    nc.tensor.matmul(
                        ps2[:, i, ksl], xa_t[:, tsl], cloa_t[:, ksl],
                        start=False, stop=True,
                    )
            if m < 2:
                # head: skip staging so the first argmaxes start ~2us earlier
                # (PSUM-direct costs +65ns each, saves the ACT-copy latency)
                for i in range(2):
                    scratch = scr_pool.tile([128, N_CLUSTERS], f32)
                    nc.vector._custom_dve(
                        argmax_op,
                        out=scratch[:],
                        in0=ps2[:, i, :],
                        accum_out=idxb[:, (2 * m + i) % BT : (2 * m + i) % BT + 1],
                    )
            else:
                stg = stage_pool.tile([128, 2, N_CLUSTERS], f32)
                nc.scalar.copy(stg[:], ps2[:])
                for i in range(2):
                    scratch = scr_pool.tile([128, N_CLUSTERS], f32)
                    nc.vector._custom_dve(
                        argmax_op,
                        out=scratch[:],
                        in0=stg[:, i, :],
                        accum_out=idxb[:, (2 * m + i) % BT : (2 * m + i) % BT + 1],
                    )
            if m % (BT // 2) == BT // 2 - 1:
                tb = m // (BT // 2)
                nc.scalar.copy(outbuf[:, tb * BT : (tb + 1) * BT], idxb[:])

        nc.sync.dma_start(out[:], outbuf[:])

    nc.compile()
    return nc


def _prep(x: np.ndarray, centers: np.ndarray):
    bf16 = ml_dtypes.bfloat16
    xt = np.ascontiguousarray(x.T)                      # [64, N] f32
    xhi = xt.astype(bf16)
    xlo = (xt - xhi.astype(np.float32)).astype(bf16)
    xpack = np.concatenate([xhi, xlo], axis=0)          # [128, N] bf16
    xa = np.empty((67, x.shape[0]), bf16)               # xhi + 3 ones rows
    xa[0:64] = xhi
    xa[64:67] = bf16(1.0)

    c2t = np.ascontiguousarray((2.0 * centers).T)       # [64, K] f32
    chi = c2t.astype(bf16)
    clo = (c2t - chi.astype(np.float32)).astype(bf16)   # [64, K] bf16
    cc = np.concatenate([chi, chi], axis=0)             # [128, K] bf16

    # -||c||^2 as a 3-term bf16 cascade, matched with the ones rows of xa
    cn = np.sum(centers.astype(np.float64) ** 2, axis=1)
    n1 = (-cn).astype(bf16)
    r1 = -cn - n1.astype(np.float64)
    n2 = r1.astype(bf16)
    n3 = (r1 - n2.astype(np.float64)).astype(bf16)
    cloa = np.concatenate(
        [clo, n1[None, :], n2[None, :], n3[None, :]], axis=0
    )                                                   # [67, K] bf16
    return xpack, xa, cc, cloa


def kernel(x: np.ndarray, centers: np.ndarray) -> np.ndarray:
    import sys
    if "/opt/trn_rl_repo" not in sys.path:
        sys.path.insert(0, "/opt/trn_rl_repo")
    from concourse.bass_utils import run_bass_kernel_spmd

    x = np.asarray(x, dtype=np.float32)
    centers = np.asarray(centers, dtype=np.float32)

    xpack, xa, cc, cloa = _prep(x, centers)

    if "nc" not in _CACHE:
        _CACHE["nc"] = _build_bass()
    nc = _CACHE["nc"]

    in_maps = []
    for c in range(N_CORES):
        sl = slice(c * PTS_PER_CORE, (c + 1) * PTS_PER_CORE)
        in_maps.append(
            {
                "xpack": np.ascontiguousarray(xpack[:, sl]),
                "xa": np.ascontiguousarray(xa[:, sl]),
                "cc": cc,
                "cloa": cloa,
            }
        )

    res = run_bass_kernel_spmd(nc, in_maps, list(range(N_CORES)))

    outs = []
    for c in range(N_CORES):
        o = res.results[c]["out"]                       # [128, N_TILES] uint32
        outs.append(np.asarray(o).astype(np.int64).T.reshape(-1))  # point t*128+p
    ids = np.concatenate(outs)
    return ids.astype(np.int32)


if __name__ == "__main__":
    rng = np.random.default_rng(0)
    x = rng.normal(size=(N_POINTS, N_FEATURES)).astype(np.float32)
    c = rng.normal(size=(N_CLUSTERS, N_FEATURES)).astype(np.float32)
    ids = kernel(x=x, centers=c)
    d = (
        np.sum(x * x, 1)[:, None]
        - 2.0 * (x @ c.T)
        + np.sum(c * c, 1)[None, :]
    )
    ref = np.argmin(np.abs(d), axis=1)
    print("mismatch:", np.mean(ids != ref))


# revision 25
# speedup vs baseline: 2.2223x; 1.6705x over previous
"""KMeans assignment kernel (retrieval_knn) for 8 Trainium2 NeuronCores.

Computes argmin_k ||x_n - c_k||^2 for x [262144, 64] f32 against centers
[1024, 64] f32, returning int32 cluster ids [262144].

argmin ||x-c||^2 == argmax s, s = 2x.c - ||c||^2.  Centers are pre-combined
on the host into PAIR sums/differences, so the PE emits, per point, the 512
values sum'_g = (s_2g + s_2g+1)/2 and diff'_g = (s_2g - s_2g+1)/2 (fp16
single-pass matmuls, 2x512 columns per tile).  The idle ACT engine computes
|diff'| (one Abs activation per tile), and ONE 2-stream custom DVE op per
tile (ARGMAXS_ANT: argmax over Src0+Src1 = sum'+|diff'| = max(s_2g, s_2g+1))
returns the winning PAIR index g* from a 512-wide scan — HALF the scan of a
1024-wide argmax.  The within-pair winner (1 bit) is resolved exactly on the
host by comparing the two candidate centers per point in fp64 (O(N) numpy).
No spill, no gather, no reduce cascades; DVE does one 512-elem op per tile.
"""

import numpy as np

N_POINTS = 262144
N_FEATURES = 64
N_CLUSTERS = 1024
N_PAIRS = N_CLUSTERS // 2               # 512
N_CORES = 8
PTS_PER_CORE = N_POINTS // N_CORES      # 32768
TILE_P = 128                            # points per tile (partition dim)
N_TILES = PTS_PER_CORE // TILE_P        # 256
BT = 8                                  # tiles per output batch

_CACHE = {}


def _register_ops():
    """Register the custom DVE ops (runtime append to dve_ops.OPS).

    ARGMAX_ANT  (row 17): accum = argmax_k Src0[k]            (kept for row
                          stability; unused by this kernel)
    ARGMAXS_ANT (row 18): accum = argmax_k (Src0[k] + Src1[k]) (last tie)
    """
    from concourse import dve_ops
    from concourse.dve_spec import (
        Spec, Src0, Src1, Idx, MaxNeg, AluOp, scan, eq, select, maxx,
    )

    if "ARGMAXS_ANT" in dve_ops._SUB_OPCODE_FOR_NAME:
        return _CACHE["argmaxs_op"]

    def _ref_argmax(in0, in1, s0, s1, imm2):
        r = np.maximum.accumulate(in0, axis=-1)
        idx = np.arange(in0.shape[-1], dtype=np.float32)
        return np.where(in0 == r, idx, -np.finfo(np.float32).max)

    def _ref_argmaxs(in0, in1, s0, s1, imm2):
        m = in0.astype(np.float32) + in1
        r = np.maximum.accumulate(m, axis=-1)
        idx = np.arange(m.shape[-1], dtype=np.float32)
        return np.where(m == r, idx, -np.finfo(np.float32).max)

    op1 = dve_ops.DveOp(
        "ARGMAX_ANT",
        Spec(
            body=select(eq(Src0, scan(AluOp.MAX, Src0)), Idx, MaxNeg),
            accum=maxx,
            reference=_ref_argmax,
        ),
        subdim=False,
        uops_sha={"v3": "d14dbf28477fed0e", "v4": "7311a447fa794d46"},
    )
    _mp = Src0 + Src1
    op2 = dve_ops.DveOp(
        "ARGMAXS_ANT",
        Spec(
            body=select(eq(_mp, scan(AluOp.MAX, _mp)), Idx, MaxNeg),
            accum=maxx,
            reference=_ref_argmaxs,
        ),
        subdim=False,
        uops_sha={"v3": "86f16b92aa28dba0", "v4": "0dff67e8a1d91028"},
    )
    for op in (op1, op2):
        dve_ops.OPS.append(op)
        dve_ops._SUB_OPCODE_FOR_NAME[op.name] = (
            dve_ops._CUSTOM_DVE_ROW_BASE + len(dve_ops.OPS) - 1
        )
        dve_ops.CUSTOM_DVE_SPECS[op.name] = op.spec
    _CACHE["argmaxs_op"] = op2
    return op2


def _build_bass():
    import concourse.bass as bass
    import concourse.bacc as bacc
    import concourse.mybir as mybir
    import concourse.tile as tile
    from contextlib import ExitStack

    argmaxs_op = _register_ops()

    f16 = mybir.dt.float16
    f32 = mybir.dt.float32
    u32 = mybir.dt.uint32

    nc = bacc.Bacc(None, target_bir_lowering=False)

    xq = nc.declare_dram_parameter("xq", [67, PTS_PER_CORE], f16, isOutput=False)
    ccs = nc.declare_dram_parameter("ccs", [67, N_PAIRS], f16, isOutput=False)
    ccd = nc.declare_dram_parameter("ccd", [67, N_PAIRS], f16, isOutput=False)
    out = nc.declare_dram_parameter("out", [128, N_TILES], u32, isOutput=True)

    with tile.TileContext(nc) as tc, ExitStack() as ctx:
        const_pool = ctx.enter_context(tc.tile_pool(name="const", bufs=1))
        psum_pool = ctx.enter_context(
            tc.tile_pool(name="psum", bufs=4, space=bass.MemorySpace.PSUM)
        )
        abs_pool = ctx.enter_context(tc.tile_pool(name="absd", bufs=4))
        scr_pool = ctx.enter_context(tc.tile_pool(name="scr", bufs=3))
        idx_pool = ctx.enter_context(tc.tile_pool(name="idx", bufs=3))
        out_pool = ctx.enter_context(tc.tile_pool(name="out", bufs=1))

        ccs_t = const_pool.tile([67, N_PAIRS], f16)
        nc.gpsimd.dma_start(ccs_t[:], ccs[:])
        ccd_t = const_pool.tile([67, N_PAIRS], f16)
        nc.gpsimd.dma_start(ccd_t[:], ccd[:])
        xq_t = const_pool.tile([67, PTS_PER_CORE], f16)
        XCH = 32
        CHW = PTS_PER_CORE // XCH
        for ch in range(XCH):
            csl = slice(ch * CHW, (ch + 1) * CHW)
            nc.sync.dma_start(xq_t[:, csl], xq[:, csl])

        # warm the PE p-state during the x-load dead time
        ps2 = psum_pool.tile([128, 2, N_PAIRS], f32)
        for _ in range(8):
            nc.tensor.matmul(
                ps2[:, 0, :], ccs_t[:, 0:TILE_P], ccs_t[:],
                start=True, stop=True,
            )

        outbuf = out_pool.tile([128, N_TILES], u32)

        for t in range(N_TILES):
            i = t % BT
            if i == 0:
                idxb = idx_pool.tile([128, BT], f32)
            tsl = slice(t * TILE_P, (t + 1) * TILE_P)
            ps2 = psum_pool.tile([128, 2, N_PAIRS], f32)
            nc.tensor.matmul(
                ps2[:, 0, :], xq_t[:, tsl], ccs_t[:], start=True, stop=True
            )
            nc.tensor.matmul(
                ps2[:, 1, :], xq_t[:, tsl], ccd_t[:], start=True, stop=True
            )
            absd = abs_pool.tile([128, N_PAIRS], f32)
            nc.scalar.activation(
                absd[:], ps2[:, 1, :], mybir.ActivationFunctionType.Abs
            )
            scratch = scr_pool.tile([128, N_PAIRS], f32)
            nc.vector._custom_dve(
                argmaxs_op,
                out=scratch[:],
                in0=ps2[:, 0, :],
                in1=absd[:],
                accum_out=idxb[:, i : i + 1],
            )
            if i == BT - 1:
                tb = t // BT
                nc.vector.tensor_copy(outbuf[:, tb * BT : (tb + 1) * BT], idxb[:])

        nc.sync.dma_start(out[:], outbuf[:])

    nc.compile()
    return nc


def _casc3(A):
    """3-row fp16 cascade summing (exactly, up to fp16 subnormal flush) to A."""
    f16 = np.float16
    n1 = A.astype(f16)
    r1 = A - n1.astype(np.float64)
    n2 = r1.astype(f16)
    n3 = (r1 - n2.astype(np.float64)).astype(f16)
    return n1, n2, n3


def _prep(x: np.ndarray, centers: np.ndarray):
    f16 = np.float16
    xd = x.astype(np.float64)
    cd = centers.astype(np.float64)

    xq = np.empty((67, N_POINTS), f16)
    xq[0:64] = np.ascontiguousarray(xd.T).astype(f16)
    xq[64:67] = f16(1.0)

    cn = (cd * cd).sum(1)
    csum = cd[0::2] + cd[1::2]                  # [512, 64]
    cdif = cd[0::2] - cd[1::2]
    cnsum = (cn[0::2] + cn[1::2]) / 2.0
    cndif = (cn[0::2] - cn[1::2]) / 2.0

    # device computes sum'_g = x.csum - cnsum = (s_2g + s_2g+1)/2
    #             and diff'_g = x.cdif - cndif = (s_2g - s_2g+1)/2
    ccs = np.empty((67, N_PAIRS), f16)
    ccs[0:64] = csum.T.astype(f16)
    ccs[64], ccs[65], ccs[66] = _casc3(-cnsum)
    ccd = np.empty((67, N_PAIRS), f16)
    ccd[0:64] = cdif.T.astype(f16)
    ccd[64], ccd[65], ccd[66] = _casc3(-cndif)
    return xq, ccs, ccd


def kernel(x: np.ndarray, centers: np.ndarray) -> np.ndarray:
    import sys
    if "/opt/trn_rl_repo" not in sys.path:
        sys.path.insert(0, "/opt/trn_rl_repo")
    from concourse.bass_utils import run_bass_kernel_spmd

    x = np.asarray(x, dtype=np.float32)
    centers = np.asarray(centers, dtype=np.float32)

    xq, ccs, ccd = _prep(x, centers)

    if "nc" not in _CACHE:
        _CACHE["nc"] = _build_bass()
    nc = _CACHE["nc"]

    in_maps = []
    for c in range(N_CORES):
        sl = slice(c * PTS_PER_CORE, (c + 1) * PTS_PER_CORE)
        in_maps.append(
            {
                "xq": np.ascontiguousarray(xq[:, sl]),
                "ccs": ccs,
                "ccd": ccd,
            }
        )

    res = run_bass_kernel_spmd(nc, in_maps, list(range(N_CORES)))

    outs = []
    for c in range(N_CORES):
        o = res.results[c]["out"]                       # [128, N_TILES] uint32
        outs.append(np.asarray(o).astype(np.int64).T.reshape(-1))  # point t*128+p
    g = np.concatenate(outs)                            # winning pair per point

    # within-pair refinement on host: exact fp64 distance compare of the two
    # candidate centers; ties pick the first (matches reference argmin)
    xd = x.astype(np.float64)
    cd = centers.astype(np.float64)
    c0 = cd[2 * g]
    c1 = cd[2 * g + 1]
    d0 = ((xd - c0) ** 2).sum(1)
    d1 = ((xd - c1) ** 2).sum(1)
    ids = np.where(d1 < d0, 2 * g + 1, 2 * g)
    return ids.astype(np.int32)


if __name__ == "__main__":
    rng = np.random.default_rng(0)
    x = rng.normal(size=(N_POINTS, N_FEATURES)).astype(np.float32)
    c = rng.normal(size=(N_CLUSTERS, N_FEATURES)).astype(np.float32)
    ids = kernel(x=x, centers=c)
    d = (
        np.sum(x * x, 1)[:, None]
        - 2.0 * (x @ c.T)
        + np.sum(c * c, 1)[None, :]
    )
    ref = np.argmin(np.abs(d), axis=1)
    print("mismatch:", np.mean(ids != ref))


# revision 26
# speedup vs baseline: 2.2405x; 1.0082x over previous
"""KMeans assignment kernel (retrieval_knn) for 8 Trainium2 NeuronCores.

Computes argmin_k ||x_n - c_k||^2 for x [262144, 64] f32 against centers
[1024, 64] f32, returning int32 cluster ids [262144].

argmin ||x-c||^2 == argmax s, s = 2x.c - ||c||^2.  Centers are pre-combined
on the host into PAIR sums/differences, so the PE emits, per point, the 512
values sum'_g = (s_2g + s_2g+1)/2 and diff'_g = (s_2g - s_2g+1)/2 (fp16
single-pass matmuls, 2x512 columns per tile).  The idle ACT engine computes
|diff'| (one Abs activation per tile), and ONE 2-stream custom DVE op per
tile (ARGMAXS_ANT: argmax over Src0+Src1 = sum'+|diff'| = max(s_2g, s_2g+1))
returns the winning PAIR index g* from a 512-wide scan — HALF the scan of a
1024-wide argmax.  The within-pair winner (1 bit) is resolved exactly on the
host by comparing the two candidate centers per point in fp64 (O(N) numpy).
No spill, no gather, no reduce cascades; DVE does one 512-elem op per tile.
"""

import numpy as np

N_POINTS = 262144
N_FEATURES = 64
N_CLUSTERS = 1024
N_PAIRS = N_CLUSTERS // 2               # 512
N_CORES = 8
PTS_PER_CORE = N_POINTS // N_CORES      # 32768
TILE_P = 128                            # points per tile (partition dim)
N_TILES = PTS_PER_CORE // TILE_P        # 256
BT = 32                                 # tiles per output batch

_CACHE = {}


def _register_ops():
    """Register the custom DVE ops (runtime append to dve_ops.OPS).

    ARGMAX_ANT  (row 17): accum = argmax_k Src0[k]            (kept for row
                          stability; unused by this kernel)
    ARGMAXS_ANT (row 18): accum = argmax_k (Src0[k] + Src1[k]) (last tie)
    """
    from concourse import dve_ops
    from concourse.dve_spec import (
        Spec, Src0, Src1, Idx, MaxNeg, AluOp, scan, eq, select, maxx,
    )

    if "ARGMAXS_ANT" in dve_ops._SUB_OPCODE_FOR_NAME:
        return _CACHE["argmaxs_op"]

    def _ref_argmax(in0, in1, s0, s1, imm2):
        r = np.maximum.accumulate(in0, axis=-1)
        idx = np.arange(in0.shape[-1], dtype=np.float32)
        return np.where(in0 == r, idx, -np.finfo(np.float32).max)

    def _ref_argmaxs(in0, in1, s0, s1, imm2):
        m = in0.astype(np.float32) + in1
        r = np.maximum.accumulate(m, axis=-1)
        idx = np.arange(m.shape[-1], dtype=np.float32)
        return np.where(m == r, idx, -np.finfo(np.float32).max)

    op1 = dve_ops.DveOp(
        "ARGMAX_ANT",
        Spec(
            body=select(eq(Src0, scan(AluOp.MAX, Src0)), Idx, MaxNeg),
            accum=maxx,
            reference=_ref_argmax,
        ),
        subdim=False,
        uops_sha={"v3": "d14dbf28477fed0e", "v4": "7311a447fa794d46"},
    )
    _mp = Src0 + Src1
    op2 = dve_ops.DveOp(
        "ARGMAXS_ANT",
        Spec(
            body=select(eq(_mp, scan(AluOp.MAX, _mp)), Idx, MaxNeg),
            accum=maxx,
            reference=_ref_argmaxs,
        ),
        subdim=False,
        uops_sha={"v3": "86f16b92aa28dba0", "v4": "0dff67e8a1d91028"},
    )
    for op in (op1, op2):
        dve_ops.OPS.append(op)
        dve_ops._SUB_OPCODE_FOR_NAME[op.name] = (
            dve_ops._CUSTOM_DVE_ROW_BASE + len(dve_ops.OPS) - 1
        )
        dve_ops.CUSTOM_DVE_SPECS[op.name] = op.spec
    _CACHE["argmaxs_op"] = op2
    return op2


def _build_bass():
    import concourse.bass as bass
    import concourse.bacc as bacc
    import concourse.mybir as mybir
    import concourse.tile as tile
    from contextlib import ExitStack

    argmaxs_op = _register_ops()

    f16 = mybir.dt.float16
    f32 = mybir.dt.float32
    u32 = mybir.dt.uint32

    nc = bacc.Bacc(None, target_bir_lowering=False)

    xq = nc.declare_dram_parameter("xq", [67, PTS_PER_CORE], f16, isOutput=False)
    ccs = nc.declare_dram_parameter("ccs", [67, N_PAIRS], f16, isOutput=False)
    ccd = nc.declare_dram_parameter("ccd", [67, N_PAIRS], f16, isOutput=False)
    out = nc.declare_dram_parameter("out", [128, N_TILES], u32, isOutput=True)

    with tile.TileContext(nc) as tc, ExitStack() as ctx:
        const_pool = ctx.enter_context(tc.tile_pool(name="const", bufs=1))
        psum_pool = ctx.enter_context(
            tc.tile_pool(name="psum", bufs=4, space=bass.MemorySpace.PSUM)
        )
        abs_pool = ctx.enter_context(tc.tile_pool(name="absd", bufs=4))
        scr_pool = ctx.enter_context(tc.tile_pool(name="scr", bufs=3))
        idx_pool = ctx.enter_context(tc.tile_pool(name="idx", bufs=3))
        out_pool = ctx.enter_context(tc.tile_pool(name="out", bufs=1))

        ccs_t = const_pool.tile([67, N_PAIRS], f16)
        nc.gpsimd.dma_start(ccs_t[:], ccs[:])
        ccd_t = const_pool.tile([67, N_PAIRS], f16)
        nc.gpsimd.dma_start(ccd_t[:], ccd[:])
        xq_t = const_pool.tile([67, PTS_PER_CORE], f16)
        XCH = 32
        CHW = PTS_PER_CORE // XCH
        for ch in range(XCH):
            csl = slice(ch * CHW, (ch + 1) * CHW)
            nc.sync.dma_start(xq_t[:, csl], xq[:, csl])

        # warm the PE p-state during the x-load dead time
        ps2 = psum_pool.tile([128, 2, N_PAIRS], f32)
        for _ in range(8):
            nc.tensor.matmul(
                ps2[:, 0, :], ccs_t[:, 0:TILE_P], ccs_t[:],
                start=True, stop=True,
            )

        outbuf = out_pool.tile([128, N_TILES], u32)

        for t in range(N_TILES):
            i = t % BT
            if i == 0:
                idxb = idx_pool.tile([128, BT], f32)
            tsl = slice(t * TILE_P, (t + 1) * TILE_P)
            ps2 = psum_pool.tile([128, 2, N_PAIRS], f32)
            nc.tensor.matmul(
                ps2[:, 0, :], xq_t[:, tsl], ccs_t[:], start=True, stop=True
            )
            nc.tensor.matmul(
                ps2[:, 1, :], xq_t[:, tsl], ccd_t[:], start=True, stop=True
            )
            absd = abs_pool.tile([128, N_PAIRS], f32)
            nc.scalar.activation(
                absd[:], ps2[:, 1, :], mybir.ActivationFunctionType.Abs
            )
            scratch = scr_pool.tile([128, N_PAIRS], f32)
            nc.vector._custom_dve(
                argmaxs_op,
                out=scratch[:],
                in0=ps2[:, 0, :],
                in1=absd[:],
                accum_out=idxb[:, i : i + 1],
            )
            if i == BT - 1:
                tb = t // BT
                nc.vector.tensor_copy(outbuf[:, tb * BT : (tb + 1) * BT], idxb[:])

        nc.sync.dma_start(out[:], outbuf[:])

    nc.compile()
    return nc


def _casc3(A):
    """3-row fp16 cascade summing (exactly, up to fp16 subnormal flush) to A."""
    f16 = np.float16
    n1 = A.astype(f16)
    r1 = A - n1.astype(np.float64)
    n2 = r1.astype(f16)
    n3 = (r1 - n2.astype(np.float64)).astype(f16)
    return n1, n2, n3


def _prep(x: np.ndarray, centers: np.ndarray):
    f16 = np.float16
    xd = x.astype(np.float64)
    cd = centers.astype(np.float64)

    xq = np.empty((67, N_POINTS), f16)
    xq[0:64] = np.ascontiguousarray(xd.T).astype(f16)
    xq[64:67] = f16(1.0)

    cn = (cd * cd).sum(1)
    csum = cd[0::2] + cd[1::2]                  # [512, 64]
    cdif = cd[0::2] - cd[1::2]
    cnsum = (cn[0::2] + cn[1::2]) / 2.0
    cndif = (cn[0::2] - cn[1::2]) / 2.0

    # device computes sum'_g = x.csum - cnsum = (s_2g + s_2g+1)/2
    #             and diff'_g = x.cdif - cndif = (s_2g - s_2g+1)/2
    ccs = np.empty((67, N_PAIRS), f16)
    ccs[0:64] = csum.T.astype(f16)
    ccs[64], ccs[65], ccs[66] = _casc3(-cnsum)
    ccd = np.empty((67, N_PAIRS), f16)
    ccd[0:64] = cdif.T.astype(f16)
    ccd[64], ccd[65], ccd[66] = _casc3(-cndif)
    return xq, ccs, ccd


def kernel(x: np.ndarray, centers: np.ndarray) -> np.ndarray:
    import sys
    if "/opt/trn_rl_repo" not in sys.path:
        sys.path.insert(0, "/opt/trn_rl_repo")
    from concourse.bass_utils import run_bass_kernel_spmd

    x = np.asarray(x, dtype=np.float32)
    centers = np.asarray(centers, dtype=np.float32)

    xq, ccs, ccd = _prep(x, centers)

    if "nc" not in _CACHE:
        _CACHE["nc"] = _build_bass()
    nc = _CACHE["nc"]

    in_maps = []
    for c in range(N_CORES):
        sl = slice(c * PTS_PER_CORE, (c + 1) * PTS_PER_CORE)
        in_maps.append(
            {
                "xq": np.ascontiguousarray(xq[:, sl]),
                "ccs": ccs,
                "ccd": ccd,
            }
        )

    res = run_bass_kernel_spmd(nc, in_maps, list(range(N_CORES)))

    outs = []
    for c in range(N_CORES):
        o = res.results[c]["out"]                       # [128, N_TILES] uint32
        outs.append(np.asarray(o).astype(np.int64).T.reshape(-1))  # point t*128+p
    g = np.concatenate(outs)                            # winning pair per point

    # within-pair refinement on host: exact fp64 distance compare of the two
    # candidate centers; ties pick the first (matches reference argmin)
    xd = x.astype(np.float64)
    cd = centers.astype(np.float64)
    c0 = cd[2 * g]
    c1 = cd[2 * g + 1]
    d0 = ((xd - c0) ** 2).sum(1)
    d1 = ((xd - c1) ** 2).sum(1)
    ids = np.where(d1 < d0, 2 * g + 1, 2 * g)
    return ids.astype(np.int32)


if __name__ == "__main__":
    rng = np.random.default_rng(0)
    x = rng.normal(size=(N_POINTS, N_FEATURES)).astype(np.float32)
    c = rng.normal(size=(N_CLUSTERS, N_FEATURES)).astype(np.float32)
    ids = kernel(x=x, centers=c)
    d = (
        np.sum(x * x, 1)[:, None]
        - 2.0 * (x @ c.T)
        + np.sum(c * c, 1)[None, :]
    )
    ref = np.argmin(np.abs(d), axis=1)
    print("mismatch:", np.mean(ids != ref))
